# revision 28
# baseline (speedup 1.0000x reference)
"""Multi-head attention (LoRA QKV + ALiBi + causal softmax + output proj) on 8 TRN2 cores.

Sharding: core = (batch b in 0..3, head-group hg in 0..1); each core handles one batch
element and 8 of the 16 heads.  LoRA is folded into effective weights on the host
(W_eff = W + 2*A@B, exact algebra).  Each core computes a partial projection output
(its 512 attention dims x full Wp rows); the host sums the two partials per batch.

On-core math (all matmuls in float32r = full PE speed for free-dim >= 256):
  qT[d,t] = sum_e wqT[e,d] * xT[e,t]          (wqT pre-scaled by 1/sqrt(dh) on host)
  kT[d,t], v[t,d] similar
  sT[j,i] = sum_d kT[d,j] qT[d,i]             (two heads packed per 64-row PE strip)
  p[j,i]  = exp(sT[j,i] - slope*j - C)        (analytic softmax max M_i = slope*i + C
                                               cancels the +slope*i ALiBi term; exact
                                               after normalization)
  causal: p[j,i] = 0 where j > i              (gpsimd affine_select on diagonal tiles)
  pv[d,i] = sum_j v'[j,d] p[j,i]              (v' has a ones column -> row d=64 is the
                                               softmax denominator)
  outT[d,i] = pv[d,i] / pv[64,i]              (reciprocal + ones-matmul broadcast)
  out[t,e] = sum_d outT[d,t] * wpT[d,e]       (partial; host adds the other head-group)

Schedule: V for all heads first; then per head-pair hp the attention c-chunks are
explicitly interleaved with the NEXT pair's qT/kT projection groups, because the PE
executes its stream in order and ACT (exp) is the attention-phase bottleneck: the
projection matmuls fill PE while ACT drains.
"""

import math
from contextlib import ExitStack

import numpy as np

import concourse.bacc as bacc
import concourse.mybir as mybir
import concourse.tile as tile
from concourse.bass_utils import run_bass_kernel_spmd

T, E, DH, H = 2048, 1024, 64, 16
HL = 8              # heads per core
NKT = 8             # contraction tiles of 128 over E
NTT = 16            # token tiles of 128 over T
CB = 12.0           # safety constant in the analytic softmax max
NEG = -1.0e30

_NC_CACHE = None


def _build_nc():
    f32 = mybir.dt.float32
    f32r = mybir.dt.float32r
    Exp = mybir.ActivationFunctionType.Exp

    nc = bacc.Bacc(trn_type="TRN2", target_bir_lowering=False, debug=False)
    xT_d = nc.declare_dram_parameter("xT", [E, T], f32, isOutput=False)
    wqT_d = nc.declare_dram_parameter("wqT", [E, 512], f32, isOutput=False)
    wkT_d = nc.declare_dram_parameter("wkT", [E, 512], f32, isOutput=False)
    wvT_d = nc.declare_dram_parameter("wvT", [E, 512], f32, isOutput=False)
    wpT_d = nc.declare_dram_parameter("wpT", [512, E], f32, isOutput=False)
    eb_d = nc.declare_dram_parameter("ebias", [128, 128], f32, isOutput=False)
    tri_d = nc.declare_dram_parameter("trineg", [128, 128], f32, isOutput=False)
    ones_d = nc.declare_dram_parameter("onesd", [128, 128], f32, isOutput=False)
    out_d = nc.declare_dram_parameter("out", [T, E], f32, isOutput=True)

    with ExitStack() as st:
        tc = st.enter_context(tile.TileContext(nc))
        ps = st.enter_context(tc.tile_pool(name="ps", bufs=1, space="PSUM"))
        # psum tags: acc(2) + s(4) + pv(2) = 8 banks exactly
        sb_r = st.enter_context(tc.tile_pool(name="sbr", bufs=1, side="right"))
        sb_x = st.enter_context(tc.tile_pool(name="sbx", bufs=1, side="left"))
        sb_l = st.enter_context(tc.tile_pool(name="sbl", bufs=1, side="left"))

        # ---------- DMA plumbing ----------
        # sync queue: xT column-chunks paced against the V stage; gpsimd queue:
        # weights + small constants, so they don't delay the xT stream.
        xts = []
        for k in range(NKT):
            xts.append(sb_x.tile([128, T], f32r, tag=f"xt{k}", bufs=1, name=f"xt{k}"))

        def dma_xt_chunk(ck):
            for k in range(NKT):
                nc.sync.dma_start(
                    out=xts[k][:, ck * 512:(ck + 1) * 512],
                    in_=xT_d[k * 128:(k + 1) * 128, ck * 512:(ck + 1) * 512].bitcast(f32r))

        dma_xt_chunk(0)
        dma_xt_chunk(1)
        wvs = []
        for k in range(NKT):
            t = sb_l.tile([128, 512], f32r, tag="wst", bufs=8, name=f"wv{k}")
            nc.gpsimd.dma_start(out=t[:], in_=wvT_d[k * 128:(k + 1) * 128, :].bitcast(f32r))
            wvs.append(t)
        gv_sb = sb_r.tile([128, 128], f32, tag="gv", bufs=1)
        ones_t = sb_r.tile([128, 64], f32r, tag="ones", bufs=1)

        qts = [None] * 4
        kts = [None] * 4
        wqk = [None] * 4
        outTs = [None] * 4

        def emit_wqk_dma(hp):
            tiles = {}
            for which, wd in (("q", wqT_d), ("k", wkT_d)):
                wt = sb_l.tile([128, 1024], f32r, tag="wqk", bufs=2,
                               name=f"w{which}{hp}")
                src = wd[:, hp * 128:(hp + 1) * 128]
                src = src.rearrange("(k p) m -> p k m", p=128).bitcast(f32r)
                nc.gpsimd.dma_start(out=wt.rearrange("p (k m) -> p k m", k=NKT), in_=src)
                tiles[which] = wt
            wqk[hp] = tiles
            qts[hp] = sb_l.tile([128, T], f32r, tag="qt", bufs=2, name=f"qt{hp}")
            kts[hp] = sb_l.tile([128, T], f32r, tag="kt", bufs=2, name=f"kt{hp}")

        def emit_qk_group(hp, which, tck):
            wt = wqk[hp][which]
            ot = qts[hp] if which == "q" else kts[hp]
            pq = ps.tile([128, 512], f32, tag="acc", bufs=2)
            for k in range(NKT):
                nc.tensor.matmul(pq[:], wt[:, k * 128:(k + 1) * 128],
                                 xts[k][:, tck * 512:(tck + 1) * 512],
                                 start=(k == 0), stop=(k == NKT - 1))
            nc.vector.tensor_copy(ot[:, tck * 512:(tck + 1) * 512], pq[:])

        vts = [None] * NTT

        def emit_v_group(tt):
            pvm = ps.tile([128, 512], f32, tag="acc", bufs=2)
            for k in range(NKT):
                nc.tensor.matmul(pvm[:], xts[k][:, tt * 128:(tt + 1) * 128], wvs[k][:],
                                 start=(k == 0), stop=(k == NKT - 1))
            vt = sb_r.tile([128, HL * 65], f32r, tag=f"v{tt}", bufs=1, name=f"v{tt}")
            v3 = vt.rearrange("p (h c) -> p h c", h=HL)
            for h in range(HL):
                nc.vector.tensor_scalar_mul(
                    v3[:, h, 0:64], pvm[:, h * 64:(h + 1) * 64],
                    gv_sb[:, tt * HL + h:tt * HL + h + 1])
            nc.vector.tensor_copy(
                v3[:, :, 64:65],
                gv_sb[:, tt * HL:(tt + 1) * HL].rearrange("p (h c) -> p h c", c=1))
            vts[tt] = vt

        wps = [None] * 8

        def emit_wp_dma():
            for i in range(8):  # i = hp*2 + ec
                hp, ec = i // 2, i % 2
                t = sb_l.tile([128, 512], f32r, tag="wst", bufs=8, name=f"wp{i}")
                nc.gpsimd.dma_start(
                    out=t[:],
                    in_=wpT_d[hp * 128:(hp + 1) * 128,
                              ec * 512:(ec + 1) * 512].bitcast(f32r))
                wps[i] = t

        def emit_proj_group(tt, ec):
            po = ps.tile([128, 512], f32, tag="acc", bufs=2)
            for hp in range(4):
                nc.tensor.matmul(po[:], outTs[hp][:, tt * 128:(tt + 1) * 128],
                                 wps[hp * 2 + ec][:], start=(hp == 0), stop=(hp == 3))
            ob = sb_l.tile([128, 512], f32, tag="ob", bufs=2)
            nc.vector.tensor_copy(ob[:], po[:])
            nc.sync.dma_start(out=out_d[tt * 128:(tt + 1) * 128,
                                        ec * 512:(ec + 1) * 512],
                              in_=ob[:])

        # ---------- filler schedule: PE work emitted between attention chunks ----
        # deadline rule: attn(hp, c) needs q_hp[c], k_hp[0..c], v[0..4c+4)
        def fill_v(tts):
            return [lambda tt=tt: emit_v_group(tt) for tt in tts]

        def fill_qk(hp, tck):
            return [lambda: emit_qk_group(hp, "q", tck),
                    lambda: emit_qk_group(hp, "k", tck)]

        fills = {}
        fills[(0, 0)] = ([lambda: dma_xt_chunk(2)] + fill_v(range(4, 8))
                         + fill_qk(0, 2))
        fills[(0, 1)] = ([lambda: dma_xt_chunk(3)] + fill_v(range(8, 12))
                         + fill_qk(0, 3))
        fills[(0, 2)] = (fill_v(range(12, 16)) + [lambda: emit_wqk_dma(1)]
                         + fill_qk(1, 0) + [emit_wp_dma])
        fills[(0, 3)] = fill_qk(1, 1)
        fills[(1, 0)] = fill_qk(1, 2)
        fills[(1, 1)] = fill_qk(1, 3)
        fills[(1, 2)] = [lambda: emit_wqk_dma(2)] + fill_qk(2, 0)
        fills[(1, 3)] = fill_qk(2, 1)
        fills[(2, 0)] = fill_qk(2, 2)
        fills[(2, 1)] = fill_qk(2, 3)
        fills[(2, 2)] = [lambda: emit_wqk_dma(3)] + fill_qk(3, 0)
        fills[(2, 3)] = fill_qk(3, 1)

        def proj_fills(c):
            return [(lambda tt=tt, ec=ec: emit_proj_group(tt, ec))
                    for tt in range(4 * c, 4 * c + 4) for ec in range(2)]

        fills[(3, 0)] = fill_qk(3, 2) + proj_fills(0)
        fills[(3, 1)] = fill_qk(3, 3) + proj_fills(1)
        fills[(3, 2)] = proj_fills(2)
        fills[(3, 3)] = proj_fills(3)

        # ---------- preloop ----------
        nc.gpsimd.dma_start(out=gv_sb[:], in_=eb_d[:])
        nc.gpsimd.dma_start(out=ones_t[:], in_=ones_d[:, 0:64].bitcast(f32r))
        for tt in range(4):
            emit_v_group(tt)
        emit_wqk_dma(0)
        for fn in fill_qk(0, 0) + fill_qk(0, 1):
            fn()

        # ---------- attention ----------
        for hp in range(4):
            qt, kt = qts[hp], kts[hp]
            h0, h1 = 2 * hp, 2 * hp + 1
            oT = sb_r.tile([128, T], f32r, tag=f"ot{hp}", bufs=1, name=f"ot{hp}")
            outTs[hp] = oT
            for slot, c in enumerate(range(4)):
                pv0 = ps.tile([128, 512], f32, tag="pv", bufs=2)
                pv1 = ps.tile([128, 512], f32, tag="pv", bufs=2)
                njt = 4 * c + 4
                for jt in range(njt):
                    r = jt - 4 * c
                    # keep the moving free-dim >= 256 (fp32r runs 4x slower
                    # below 256): widen the r=3 chunk; extra columns are
                    # fully masked by a wider affine_select window
                    cw = max(512 - 128 * r, 256) if r > 0 else 512
                    mw = 128 * r - (512 - cw) + 128 if r > 0 else 128
                    ioff = c * 512 + (512 - cw)
                    s01 = ps.tile([128, 1024], f32, tag="s", bufs=2)
                    nc.tensor.matmul(s01[:, 0:cw], kt[0:64, jt * 128:(jt + 1) * 128],
                                     qt[0:64, ioff:ioff + cw], start=True, stop=True)
                    nc.tensor.matmul(s01[:, 512:512 + cw],
                                     kt[64:128, jt * 128:(jt + 1) * 128],
                                     qt[64:128, ioff:ioff + cw], start=True, stop=True)
                    p01 = sb_l.tile([128, 1024], f32r, tag="pt", bufs=2)
                    s3 = s01.rearrange("p (h m) -> p h m", h=2)
                    p3 = p01.rearrange("p (h m) -> p h m", h=2)
                    nc.scalar.activation(p3[:, :, 0:cw], s3[:, :, 0:cw], Exp)
                    if r >= 0:
                        # zero the j > i region at the head of the window:
                        # keep where (i - j) = (m - (mw - 128)) - pj >= 0
                        for off in (0, 512):
                            nc.gpsimd.affine_select(
                                out=p01[:, off:off + mw], in_=p01[:, off:off + mw],
                                compare_op=mybir.AluOpType.is_ge, fill=0.0,
                                base=-(mw - 128), pattern=[[1, mw]],
                                channel_multiplier=-1)
                    nc.tensor.matmul(pv0[0:65, 512 - cw:512],
                                     vts[jt][:, h0 * 65:h0 * 65 + 65], p01[:, 0:cw],
                                     start=(jt == 0), stop=(jt == njt - 1))
                    nc.tensor.matmul(pv1[0:65, 512 - cw:512],
                                     vts[jt][:, h1 * 65:h1 * 65 + 65],
                                     p01[:, 512:512 + cw],
                                     start=(jt == 0), stop=(jt == njt - 1))
                # normalize: outT[d, i] = pv[d, i] * (1 / pv[64, i])
                for par, pvx in ((0, pv0), (1, pv1)):
                    rr = sb_l.tile([65, 512], f32r, tag="rr", bufs=2)
                    with nc.allow_low_precision("f32r reciprocal of softmax denom"):
                        nc.vector.reciprocal(rr[64:65, :], pvx[64:65, :])
                    bp = ps.tile([64, 512], f32, tag="acc", bufs=2)
                    nc.tensor.matmul(bp[0:64, :], ones_t[64:65, 0:64], rr[64:65, :],
                                     start=True, stop=True)
                    bb = sb_l.tile([64, 512], f32r, tag="bb", bufs=2)
                    nc.vector.tensor_copy(bb[:], bp[0:64, :])
                    if par == 0:
                        nc.vector.tensor_mul(oT[0:64, c * 512:(c + 1) * 512],
                                             pvx[0:64, :], bb[:])
                    else:
                        tm = sb_l.tile([64, 512], f32r, tag="tm", bufs=1)
                        nc.vector.tensor_mul(tm[:], pvx[0:64, :], bb[:])
                        nc.sync.dma_start(out=oT[64:128, c * 512:(c + 1) * 512],
                                          in_=tm[:])
                # PE fillers: next projection groups / V tiles / output proj
                for fn in fills.get((hp, slot), []):
                    fn()

    nc.finalize()
    return nc


def _get_nc():
    global _NC_CACHE
    if _NC_CACHE is None:
        _NC_CACHE = _build_nc()
    return _NC_CACHE


def _slopes():
    start = 2.0 ** (-(2.0 ** (-(math.log2(H) - 3.0))))
    return np.array([start * start ** i for i in range(H)], dtype=np.float64)


def _host_prep(x, Wq, Aq, Bq, Wk, Ak, Bk, Wv, Av, Bv, Wp):
    f8 = np.float64
    weff = {}
    for nm, W, A, B in (("q", Wq, Aq, Bq), ("k", Wk, Ak, Bk), ("v", Wv, Av, Bv)):
        weff[nm] = (W.astype(f8) + 2.0 * (A.astype(f8) @ B.astype(f8)))
    weff["q"] = weff["q"] / math.sqrt(DH)          # fold 1/sqrt(dh) into q weights
    slopes = _slopes()

    pj = np.arange(128)
    pi = np.arange(128)
    trineg = np.where(pj[:, None] <= pi[None, :], 0.0, NEG).astype(np.float32)
    jj = np.arange(T, dtype=np.float64).reshape(16, 128).T   # [pj, jt] -> j

    in_maps = []
    for b in range(4):
        xT = np.ascontiguousarray(x[b].T)
        for hg in range(2):
            S = slice(hg * 512, hg * 512 + 512)
            # gv[pj, tt*8 + h] = exp(-(slope_h * j + C)), j = tt*128 + pj
            gv = np.stack([np.exp(-(slopes[hg * 8 + hl] * jj + CB))
                           for hl in range(HL)], axis=2)   # [128, 16, 8]
            gv = gv.reshape(128, 16 * HL).astype(np.float32)
            in_maps.append({
                "xT": xT,
                "wqT": np.ascontiguousarray(weff["q"][S].T).astype(np.float32),
                "wkT": np.ascontiguousarray(weff["k"][S].T).astype(np.float32),
                "wvT": np.ascontiguousarray(weff["v"][S].T).astype(np.float32),
                "wpT": np.ascontiguousarray(Wp[:, S].T),
                "ebias": gv,
                "trineg": trineg,
                "onesd": np.ones((128, 128), dtype=np.float32),
            })
    return in_maps


def run(inputs, trace=False):
    nc = _get_nc()
    inputs = {k: np.asarray(v, dtype=np.float32) for k, v in inputs.items()}
    in_maps = _host_prep(**inputs)
    res = run_bass_kernel_spmd(nc, in_maps, list(range(8)), trace=trace)
    outs = [np.asarray(res.results[i]["out"]) for i in range(8)]
    full = np.stack([outs[2 * b] + outs[2 * b + 1] for b in range(4)])
    return full.astype(np.float32), res


def kernel(**inputs):
    full, _ = run(inputs, trace=False)
    return full
